# revision 12
# baseline (speedup 1.0000x reference)
"""MultiHeadAttention Trainium2 Bass kernel.

Problem: N=4, S=2048, EMBED=512, HEADS=8, HEAD_DIM=64, fp32.
  v = (values.r(N,S,H,D) @ Wv.T); k = ...Wk.T; q = ...Wq.T
  energy = einsum('nqhd,nkhd->nhqk', q, k)/8; attn = softmax(energy, -1)
  out = einsum('nhql,nlhd->nqhd', attn, v).r(N,S,E) @ Wo.T + bo
(mask is all-ones per the input spec -> identity; not applied on device)

Sharding: 8 cores = 4 batches x 2 query-halves. Each core computes all 8
heads for its (batch, 1024-query) slice and the final fc_out rows -> no
cross-core communication; host just concatenates slices.

Per-core algorithm (keeps everything fp32; matmuls run as float32r which
streams at 1 cycle/row when the moving dim is >= 256):
  - xk/xq are PE-transposed on chip to [d, s] layout (DMA transpose is
    2-byte only). xv is loaded naturally with a ones column appended per
    head: the attention*V matmul then yields softmax denominators free.
  - Wk is folded into the query side: energy^T = xk @ (xq @ Wqk)^T with
    Wqk = Wq^T Wk computed on chip, so raw transposed keys are the
    stationary operand (no k projection).
  - Wv is folded past attention: Z = xv_aug^T-contraction with exp(E),
    then attn_outT = diag(Wv^T, Wv^T) @ Z_normalized.
  - softmax: energy tiles [128k, 3, 512q] in PSUM, exp'd by single ACT
    instructions (1536 wide) into SBUF; no max subtraction (logits are
    ~N(0,1), |logit| < ~6).
  - Normalization: denominator rows are PE-transposed to token-major,
    reciprocal on DVE, transposed back, partition-broadcast on GPSIMD,
    applied with one tensor_mul per head.
  - fc_out: Wo transposed on chip (16 PE transposes); out = attn_outT^T
    blocks @ WoT + bo (bias broadcast-loaded).
"""

import sys

if "/opt/trn_rl_repo" not in sys.path:
    sys.path.insert(0, "/opt/trn_rl_repo")

import numpy as np

import concourse.bass as bass
import concourse.mybir as mybir
import concourse.tile as tile
from concourse import bacc
from concourse.bass_utils import run_bass_kernel_spmd
from concourse.masks import make_identity

F32 = mybir.dt.float32
F32R = mybir.dt.float32r

N_BATCH = 4
S = 2048
E = 512
H = 8
D = 64
SQ = 1024  # queries per core
P = 128
NKT = S // P  # 16 k-tiles
NQB = SQ // 512  # q blocks of 512
NPAIR = 4  # head pairs
TG = 3  # k-tiles per exp group (3 PSUM banks)


def _r(ap):
    return ap.bitcast(F32R)


# Tiles consumed by float32r matmuls are declared float32r natively: the
# BIR verifier requires every producer to round-on-write into the format.


def _phase_a(nc, tc, const, bigT, vstage, nat, psA, dram):
    """Constants, weight prep, input transposes, q2 projection."""
    xq, xk, xv, wq, wk, wv, wo, bo = dram

    ident = const.tile([P, P], F32)
    make_identity(nc, ident)

    bo_b = const.tile([P, E], F32)
    nc.sync.dma_start(out=bo_b, in_=bo[None, :].to_broadcast((P, E)))

    wq_s = const.tile([D, D], F32, tag="wsmall_q")
    wk_s = const.tile([D, D], F32, tag="wsmall_k")
    wv_s = const.tile([D, D], F32, tag="wsmall_v")
    nc.sync.dma_start(out=wq_s, in_=wq[:, :])
    nc.sync.dma_start(out=wk_s, in_=wk[:, :])
    nc.sync.dma_start(out=wv_s, in_=wv[:, :])

    ones_col = const.tile([P, 1], F32, tag="ones_col")
    nc.vector.memset(ones_col, 1.0)

    # Wqk = Wq^T @ Wk  [d_in(q), d_in(k)] ; diag-doubled for head pairs.
    # (memset cannot write float32r -> build in f32 staging, round-copy.)
    wqk_p = psA.tile([D, D], F32, tag="psmall")
    nc.tensor.matmul(wqk_p, wq_s, wk_s)
    dstage = const.tile([P, P], F32, tag="dstage")
    nc.vector.memset(dstage, 0.0)
    nc.vector.tensor_copy(dstage[0:D, 0:D], wqk_p)
    nc.vector.tensor_copy(dstage[D:P, D:P], wqk_p)
    qkw_diag = const.tile([P, P], F32R, tag="qkw_diag")
    nc.vector.tensor_copy(qkw_diag, dstage)

    # WvT diag tile
    wvT_p = psA.tile([D, D], F32, tag="psmall")
    nc.tensor.transpose(wvT_p, wv_s, ident[0:D, 0:D])
    dstage2 = const.tile([P, P], F32, tag="dstage2")
    nc.vector.memset(dstage2, 0.0)
    nc.vector.tensor_copy(dstage2[0:D, 0:D], wvT_p)
    nc.vector.tensor_copy(dstage2[D:P, D:P], wvT_p)
    wv_diag = const.tile([P, P], F32R, tag="wv_diag")
    nc.vector.tensor_copy(wv_diag, dstage2)

    # WoT[:, c, :] = Wo^T rows [128c:128c+128] (e_in major)
    woT = const.tile([P, 4, E], F32R)
    for rr in range(4):
        wo_nat = nat.tile([P, E], F32, tag="wo_nat")
        nc.sync.dma_start(out=wo_nat, in_=wo[P * rr : P * (rr + 1), :])
        for cc in range(4):
            tp = psA.tile([P, P], F32, tag="ptrans")
            nc.tensor.transpose(tp, wo_nat[:, P * cc : P * (cc + 1)], ident)
            nc.vector.tensor_copy(woT[:, cc, P * rr : P * (rr + 1)], tp)

    # ---------- load + transpose xk, xq; stage xv ----------
    xkT = [bigT.tile([P, S], F32R, tag=f"xkT{p}", name=f"xkT{p}")
           for p in range(NPAIR)]
    q2T = [bigT.tile([P, SQ], F32R, tag=f"q2T{p}", name=f"q2T{p}")
           for p in range(NPAIR)]
    xvs = [vstage.tile([P, H, D + 2], F32R, tag=f"xvs{st}", name=f"xvs{st}")
           for st in range(NKT)]

    for st in range(NKT):
        xk_nat = nat.tile([P, E], F32, tag="xk_nat")
        nc.sync.dma_start(out=xk_nat, in_=xk[P * st : P * (st + 1), :])
        for p in range(NPAIR):
            tp = psA.tile([P, P], F32, tag="ptrans")
            nc.tensor.transpose(tp, xk_nat[:, P * p : P * (p + 1)], ident)
            nc.vector.tensor_copy(xkT[p][:, P * st : P * (st + 1)], tp)

        xv_nat = nat.tile([P, E], F32, tag="xv_nat")
        nc.sync.dma_start(out=xv_nat, in_=xv[P * st : P * (st + 1), :])
        nc.vector.tensor_copy(
            out=xvs[st][:, :, 0:D],
            in_=xv_nat.rearrange("p (h d) -> p h d", h=H),
        )
        nc.vector.tensor_copy(
            out=xvs[st][:, :, D : D + 1],
            in_=ones_col[:, None, :].to_broadcast((P, H, 1)),
        )

    with tc.tile_pool(name="xqTp", bufs=1) as xqTp:
        xqT = [xqTp.tile([P, SQ], F32R, tag=f"xqT{p}", name=f"xqT{p}")
               for p in range(NPAIR)]
        _phase_a_q(nc, tc, nat, psA, xq, xqT, qkw_diag, q2T, ident)

    return ident, ones_col, bo_b, wv_diag, woT, xkT, q2T, xvs


def _phase_a_q(nc, tc, nat, psA, xq, xqT, qkw_diag, q2T, ident):
    for st in range(SQ // P):
        xq_nat = nat.tile([P, E], F32, tag="xq_nat")
        nc.sync.dma_start(out=xq_nat, in_=xq[P * st : P * (st + 1), :])
        for p in range(NPAIR):
            tp = psA.tile([P, P], F32, tag="ptrans")
            nc.tensor.transpose(tp, xq_nat[:, P * p : P * (p + 1)], ident)
            nc.vector.tensor_copy(xqT[p][:, P * st : P * (st + 1)], tp)

    # q2 = xq @ Wqk  (projected queries, transposed layout)
    for p in range(NPAIR):
        for qc in range(SQ // 512):
            q2_p = psA.tile([P, 512], F32, tag="pproj")
            nc.tensor.matmul(
                q2_p, qkw_diag, xqT[p][:, 512 * qc : 512 * (qc + 1)]
            )
            nc.vector.tensor_copy(q2T[p][:, 512 * qc : 512 * (qc + 1)], q2_p)


def build_kernel(nc):
    xq = nc.dram_tensor("xq", [SQ, E], F32, kind="ExternalInput")
    xk = nc.dram_tensor("xk", [S, E], F32, kind="ExternalInput")
    xv = nc.dram_tensor("xv", [S, E], F32, kind="ExternalInput")
    wq = nc.dram_tensor("wq", [D, D], F32, kind="ExternalInput")
    wk = nc.dram_tensor("wk", [D, D], F32, kind="ExternalInput")
    wv = nc.dram_tensor("wv", [D, D], F32, kind="ExternalInput")
    wo = nc.dram_tensor("wo", [E, E], F32, kind="ExternalInput")
    bo = nc.dram_tensor("bo", [E], F32, kind="ExternalInput")
    out = nc.dram_tensor("out", [SQ, E], F32, kind="ExternalOutput")

    groups = []
    k0 = 0
    while k0 < NKT:
        groups.append((k0, min(k0 + TG, NKT)))
        k0 += TG

    with tile.TileContext(nc) as tc:
        with (
            tc.tile_pool(name="const", bufs=1) as const,
            tc.tile_pool(name="bigT", bufs=1) as bigT,
            tc.tile_pool(name="vstage", bufs=1) as vstage,
            tc.tile_pool(name="nat", bufs=2) as nat,
            tc.tile_pool(name="work", bufs=3) as work,
        ):
            with tc.tile_pool(name="psA", bufs=2, space="PSUM") as psA:
                (ident, ones_col, bo_b, wv_diag, woT, xkT, q2T,
                 xvs) = _phase_a(
                    nc, tc, const, bigT, vstage, nat, psA,
                    (xq, xk, xv, wq, wk, wv, wo, bo),
                )

            # ---------- attention + fc ----------
            with (
                tc.tile_pool(name="psATT", bufs=2, space="PSUM") as ps,
                tc.tile_pool(name="expp", bufs=4) as expp,
                tc.tile_pool(name="zsb", bufs=9) as zsb,
                tc.tile_pool(name="small", bufs=2) as small,
                tc.tile_pool(name="bcp", bufs=4) as bcp,
                tc.tile_pool(name="znp", bufs=4) as znp,
                tc.tile_pool(name="fcl", bufs=1) as fclp,
            ):
                fcl = [fclp.tile([P, NQB, 512], F32R, tag=f"fcl{p}",
                                 name=f"fcl{p}") for p in range(NPAIR)]
                for qb in range(NQB):
                    zs_tiles = []
                    for h in range(H):
                        pair, hh = h // 2, h % 2
                        rlo, rhi = D * hh, D * hh + D
                        z_p = ps.tile([D + 1, 512], F32, tag="z")
                        for g0, g1 in groups:
                            gn = g1 - g0
                            en = ps.tile([P, TG, 512], F32, tag="energy")
                            for t in range(gn):
                                kt = g0 + t
                                nc.tensor.matmul(
                                    en[:, t, :],
                                    xkT[pair][rlo:rhi, P * kt : P * (kt + 1)],
                                    q2T[pair][rlo:rhi, 512 * qb : 512 * (qb + 1)],
                                )
                            ex = expp.tile([P, TG, 512], F32R, tag="exp")
                            nc.scalar.activation(
                                ex[:, 0:gn, :],
                                en[:, 0:gn, :],
                                mybir.ActivationFunctionType.Exp,
                                scale=0.125,
                            )
                            for t in range(gn):
                                kt = g0 + t
                                nc.tensor.matmul(
                                    z_p,
                                    xvs[kt][:, h, 0 : D + 1],
                                    ex[:, t, :],
                                    start=(kt == 0),
                                    stop=(kt == NKT - 1),
                                )
                        zs = zsb.tile([D + 1, 512], F32, tag="zs")
                        nc.vector.tensor_copy(zs, z_p)
                        zs_tiles.append(zs)

                    # Normalize + unproject per pair. Z' row D holds
                    # sum(exp) per query (free dim). Per 128-chunk:
                    # PE-transpose the row to a column (base partition 0),
                    # reciprocal on DVE, transpose back, rebuild a [1, 512]
                    # row, then GPSIMD-broadcast it across partitions.
                    # (partition_broadcast needs a base-0 source on HW.)
                    for p in range(NPAIR):
                        zn = znp.tile([P, 512], F32R, tag="zn")
                        for hh in range(2):
                            h = 2 * p + hh
                            rrow = small.tile([1, 512], F32, tag="rrow",
                                              name=f"rrow{h}", bufs=3)
                            for c in range(4):
                                csl = slice(P * c, P * (c + 1))
                                ct = ps.tile([P, 1], F32, tag="energy")
                                nc.tensor.transpose(
                                    ct, zs_tiles[h][D : D + 1, csl],
                                    ones_col[D : D + 1, 0:1],
                                )
                                rc = small.tile([P, 1], F32, tag="rc")
                                nc.vector.reciprocal(rc, ct)
                                rt = ps.tile([1, P], F32, tag="energy")
                                nc.tensor.transpose(rt, rc, ident)
                                nc.vector.tensor_copy(rrow[:, csl], rt)
                            bc = bcp.tile([D, 512], F32, tag="bc")
                            nc.gpsimd.partition_broadcast(bc, rrow[0:1, :])
                            nc.vector.tensor_mul(
                                zn[D * hh : D * hh + D, :], zs_tiles[h][0:D, :], bc
                            )
                        up = ps.tile([P, 512], F32, tag="energy")
                        nc.tensor.matmul(up, wv_diag, zn)
                        nc.vector.tensor_copy(fcl[p][:, qb, :], up)

                # ---------- fc_out ----------
                for tt in range(SQ // P):
                    qb, ti = divmod(tt, 512 // P)
                    tsl = slice(P * ti, P * (ti + 1))
                    fcp = ps.tile([P, E], F32, tag="energy")
                    for p in range(NPAIR):
                        nc.tensor.matmul(
                            fcp,
                            fcl[p][:, qb, tsl],
                            woT[:, p, :],
                            start=(p == 0),
                            stop=(p == NPAIR - 1),
                        )
                    ot = work.tile([P, E], F32, tag="ot")
                    nc.vector.tensor_add(ot, fcp, bo_b)
                    nc.sync.dma_start(out=out[P * tt : P * (tt + 1), :], in_=ot)
    return nc


_CACHED_NC = None


def _get_nc():
    global _CACHED_NC
    if _CACHED_NC is None:
        nc = bacc.Bacc(None, target_bir_lowering=False)
        build_kernel(nc)
        nc.compile()
        _CACHED_NC = nc
    return _CACHED_NC


def run_sharded(values, keys, query, Wv, Wk, Wq, Wo, bo, **spmd_kwargs):
    """Shard, run on 8 cores, gather. Returns (out, BassKernelResults)."""
    values = np.ascontiguousarray(values, dtype=np.float32)
    keys = np.ascontiguousarray(keys, dtype=np.float32)
    query = np.ascontiguousarray(query, dtype=np.float32)
    Wv = np.ascontiguousarray(Wv, dtype=np.float32)
    Wk = np.ascontiguousarray(Wk, dtype=np.float32)
    Wq = np.ascontiguousarray(Wq, dtype=np.float32)
    Wo = np.ascontiguousarray(Wo, dtype=np.float32)
    bo = np.ascontiguousarray(bo, dtype=np.float32)

    nc = _get_nc()
    in_maps = []
    for c in range(8):
        n, qh = divmod(c, 2)
        in_maps.append(
            {
                "xq": query[n, SQ * qh : SQ * (qh + 1), :],
                "xk": keys[n],
                "xv": values[n],
                "wq": Wq,
                "wk": Wk,
                "wv": Wv,
                "wo": Wo,
                "bo": bo,
            }
        )
    res = run_bass_kernel_spmd(nc, in_maps, core_ids=list(range(8)),
                               **spmd_kwargs)
    out = np.empty((N_BATCH, S, E), dtype=np.float32)
    for c in range(8):
        n, qh = divmod(c, 2)
        out[n, SQ * qh : SQ * (qh + 1), :] = res.results[c]["out"]
    return out, res


def kernel(values, keys, query, mask, Wv, Wk, Wq, Wo, bo):
    out, _ = run_sharded(values, keys, query, Wv, Wk, Wq, Wo, bo)
    return out
